# revision 10
# baseline (speedup 1.0000x reference)
"""Trainium2 Bass kernel for nn_ConvexReLU.

Math: out[i,m] = sum_{j,k,l} G[j,k] * x[i,k,l] * (v-w)[j,l,m]

Reassociated as:
    d = v - w                              (host, elementwise)
    T[k,l,m]   = sum_j G[j,k] * d[j,l,m]   (device matmul, 68.7 GFLOP)
    out[i,m]   = sum_{k,l} x[i,k,l] * T[k,l,m]   (device matmul, 17.2 GFLOP)

Sharding: split l (in_dim, 256) across 8 cores (32 each). Each core computes
a full-shape (out_dim, batch) partial; host sums the 8 partials.

Device layout per core:
    g  : (1024 j, 1024 k)        full G, replicated
    d  : (1024 j, 32 l, 128 m)   l-shard of v-w
    xt : (32 l, 128 p, 8 kt, 256 i)  l-shard of x, pre-transposed on host so
                                 each l's tile is contiguous per partition
    out: (128 m, 256 i)          partial of out^T

DMA plan (two HWDGE rings, FIFO each):
    sync  : g chunks (pg=0's critical path), then d for pg=1..7
    scalar: d for pg=0 (per-jc chunks), then ALL x tiles up front —
            paced by xpool buffer reuse, so the last l-group's x lands
            ~40us before its stage-2 instead of being fetched at the end.

Default dtype is bf16 (PE multiplies at fp22 internally, accumulates fp32;
measured rel err ~3e-3).
"""

import os
import sys

import numpy as np

for _p in ("/opt/trn_rl_repo", "/root/.axon_site/_ro/trn_rl_repo"):
    if os.path.isdir(_p) and _p not in sys.path:
        sys.path.insert(0, _p)

import concourse.bass as bass
import concourse.bacc as bacc
import concourse.mybir as mybir
from concourse.bass_utils import run_bass_kernel_spmd
from concourse.tile import TileContext

B, J, K, L, M = 256, 1024, 1024, 256, 128
NCORES = 8
LC = L // NCORES          # 32 l-values per core
NPG = 8                   # l-groups per core
LG = LC // NPG            # 4 l-values per group
NKT = K // 128            # 8 k-tiles
NJC = J // 128            # 8 j-chunks

F32 = mybir.dt.float32
F32R = mybir.dt.float32r
BF16 = mybir.dt.bfloat16

DTYPE = os.environ.get("BASS_KERNEL_DTYPE", "bf16")


def _dtypes(dtype_name: str):
    if dtype_name == "bf16":
        return BF16, BF16
    if dtype_name == "mixed":
        return F32R, BF16
    return F32R, F32R


def build_nc(dtype_name: str = DTYPE) -> bass.Bass:
    gd_dt, s2_dt = _dtypes(dtype_name)

    nc = bacc.Bacc(None, debug=False)

    g = nc.declare_dram_parameter("g", [J, K], gd_dt, isOutput=False)
    d = nc.declare_dram_parameter("d", [J, LC, M], gd_dt, isOutput=False)
    xt = nc.declare_dram_parameter("xt", [LC, 128, NKT, B], s2_dt, isOutput=False)
    out = nc.declare_dram_parameter("out", [M, B], F32, isOutput=True)

    g_r = g.rearrange("(jc p) k -> p jc k", p=128)
    d_r = d.rearrange("(jc p) l m -> p jc (l m)", p=128)

    with TileContext(nc) as tc:
        with (
            tc.tile_pool(name="gpool", bufs=1) as gpool,
            tc.tile_pool(name="dpool", bufs=3) as dpool,
            tc.tile_pool(name="tpool", bufs=3) as tpool,
            tc.tile_pool(name="xpool", bufs=12) as xpool,
            tc.tile_pool(name="opool", bufs=1) as opool,
            tc.tile_pool(name="wupool", bufs=1) as wupool,
            tc.tile_pool(name="ps1", bufs=7, space="PSUM") as ps1,
            tc.tile_pool(name="pso", bufs=1, space="PSUM") as pso,
        ):
            # ---- front DMAs: g on sync, d(pg=0) on scalar. Small first
            # chunks for a fast first matmul, then coarse chunks: each DMA
            # pays ~2us completion latency and the per-engine semaphore-lane
            # rotation is only ~4 deep, so fewer/bigger transfers keep the
            # feed ahead of the PE ----
            g_sb = gpool.tile([128, NJC, K], gd_dt)
            d_sb0 = dpool.tile([128, NJC, LG * M], gd_dt, tag="d")
            nc.sync.dma_start(out=g_sb[:, 0, 0:256], in_=g_r[:, 0, 0:256])
            nc.scalar.dma_start(out=d_sb0[:, 0, :], in_=d_r[:, 0, 0 : LG * M])
            nc.sync.dma_start(out=g_sb[:, 0, 256:], in_=g_r[:, 0, 256:])
            nc.scalar.dma_start(out=d_sb0[:, 1, :], in_=d_r[:, 1, 0 : LG * M])
            nc.sync.dma_start(out=g_sb[:, 1, :], in_=g_r[:, 1, :])
            nc.scalar.dma_start(
                out=d_sb0[:, 2:4, :], in_=d_r[:, 2:4, 0 : LG * M]
            )
            nc.sync.dma_start(out=g_sb[:, 2:4, :], in_=g_r[:, 2:4, :])
            nc.scalar.dma_start(
                out=d_sb0[:, 4:, :], in_=d_r[:, 4:, 0 : LG * M]
            )
            nc.sync.dma_start(out=g_sb[:, 4:, :], in_=g_r[:, 4:, :])

            # ---- d for pg>=1 on sync, one DMA each; dpool pacing keeps
            # at most 2 groups ahead of stage-1 consumption ----
            d_sbs = [d_sb0]
            for pg in range(1, NPG):
                d_sb = dpool.tile([128, NJC, LG * M], gd_dt, tag="d")
                nc.sync.dma_start(
                    out=d_sb[:],
                    in_=d_r[:, :, pg * LG * M : (pg + 1) * LG * M],
                )
                d_sbs.append(d_sb)

            # ---- ALL x tiles on scalar ring, issued now; xpool bufs=12
            # means at most 3 l-groups are in flight — the ring stalls on the
            # pool-reuse semaphore, which is exactly the pacing we want ----
            x_tiles = {}
            for pg in range(NPG):
                for dl in range(LG):
                    x_sb = xpool.tile(
                        [128, NKT, B], s2_dt, tag="x", name=f"x_{pg}_{dl}"
                    )
                    nc.scalar.dma_start(out=x_sb[:], in_=xt[pg * LG + dl])
                    x_tiles[(pg, dl)] = x_sb

            # ---- HAM warmup: the PE sits idle from the end of its preamble
            # (~6us) until the first DMA lands (~10.8us), and runs at the
            # K=4/8 half-clock gate for its first few us of matmuls. Filling
            # the DMA-wait window with matmuls on a memset tile banks the
            # warmup credit so real matmuls start at full clock. ----
            wu_sb = wupool.tile([128, 512], gd_dt, name="wu")
            nc.vector.memset(wu_sb[:], 1.0)
            wu_ps = ps1.tile([128, LG * M], F32, tag="p1", name="wups")
            for i in range(10):
                nc.tensor.matmul(
                    wu_ps[:],
                    wu_sb[:, 0:128],
                    wu_sb[:],
                    start=(i == 0),
                    stop=(i == 9),
                    skip_group_check=True,
                )

            out_ps = pso.tile([M, B], F32)

            total_mm2 = NPG * LG * NKT
            # kt-groups per stage-1 pass: 6 then 2 stage-1 psum banks live,
            # plus 1 out bank <= 8
            KGROUPS = [(0, 6), (6, 2)]
            KH = 4  # stage-2 kt-group width

            mm2_state = [0]

            def stage2(pg, t_sb):
                # out^T += T^T-slices @ x^T-slices for l-group pg.
                for half in range(NKT // KH):
                    for dl in range(LG):
                        for kt2 in range(KH):
                            kt = half * KH + kt2
                            nc.tensor.matmul(
                                out_ps[:],
                                t_sb[:, kt, dl * M : (dl + 1) * M],
                                x_tiles[(pg, dl)][:, kt, :],
                                start=(mm2_state[0] == 0),
                                stop=(mm2_state[0] == total_mm2 - 1),
                                skip_group_check=True,
                            )
                            mm2_state[0] += 1

            prev = None  # (pg, t_sb) whose stage-2 is pending

            for pg in range(NPG):
                # ---- stage 1: T[k, (l,m)] for this l-group ----
                d_sb = d_sbs[pg]
                t_sb = tpool.tile([128, NKT, LG * M], s2_dt, tag="t")
                for gi, (k0, kn) in enumerate(KGROUPS):
                    p1s = [
                        ps1.tile(
                            [128, LG * M], F32, tag="p1", name=f"p1_{pg}_{gi}_{i}"
                        )
                        for i in range(kn)
                    ]
                    # jc-outer: each (g[jc], d[jc]) pair is fully consumed as
                    # soon as its DMA lands
                    for jc in range(NJC):
                        for kt2 in range(kn):
                            kt = k0 + kt2
                            nc.tensor.matmul(
                                p1s[kt2][:],
                                g_sb[:, jc, kt * 128 : (kt + 1) * 128],
                                d_sb[:, jc, :],
                                start=(jc == 0),
                                stop=(jc == NJC - 1),
                                skip_group_check=True,
                            )
                    for kt2 in range(kn):
                        kt = k0 + kt2
                        nc.vector.tensor_copy(out=t_sb[:, kt, :], in_=p1s[kt2][:])

                # stage-2 lags stage-1 by one l-group
                if prev is not None:
                    stage2(*prev)
                prev = (pg, t_sb)

            stage2(*prev)

            out_sb = opool.tile([M, B], F32)
            nc.vector.tensor_copy(out=out_sb[:], in_=out_ps[:])
            nc.sync.dma_start(out=out[:], in_=out_sb[:])

    nc.finalize()
    return nc


_NC_CACHE: dict[str, bass.Bass] = {}


def _get_nc(dtype_name: str = DTYPE) -> bass.Bass:
    if dtype_name not in _NC_CACHE:
        _NC_CACHE[dtype_name] = build_nc(dtype_name)
    return _NC_CACHE[dtype_name]


def make_in_maps(x, G, v, w, dtype_name: str = DTYPE):
    x = np.asarray(x, dtype=np.float32)
    G = np.asarray(G, dtype=np.float32)
    v = np.asarray(v, dtype=np.float32)
    w = np.asarray(w, dtype=np.float32)

    d_full = v - w  # (J, L, M)

    import ml_dtypes

    if dtype_name == "bf16":
        gd_np, x_np = ml_dtypes.bfloat16, ml_dtypes.bfloat16
    elif dtype_name == "mixed":
        gd_np, x_np = np.float32, ml_dtypes.bfloat16
    else:
        gd_np, x_np = np.float32, np.float32

    G_io = np.ascontiguousarray(G.astype(gd_np))
    in_maps = []
    for c in range(NCORES):
        ls = slice(c * LC, (c + 1) * LC)
        d_c = np.ascontiguousarray(d_full[:, ls, :].astype(gd_np))
        # x (B,K,L) -> xt (LC, 128, NKT, B): xt[l, p, kt, i] = x[i, kt*128+p, l]
        xt_c = (
            x[:, :, ls]
            .transpose(2, 1, 0)                    # (LC, K, B)
            .reshape(LC, NKT, 128, B)
            .transpose(0, 2, 1, 3)                 # (LC, 128, NKT, B)
        )
        xt_c = np.ascontiguousarray(xt_c.astype(x_np))
        in_maps.append({"g": G_io, "d": d_c, "xt": xt_c})
    return in_maps


def kernel(x, G, v, w):
    nc = _get_nc()
    in_maps = make_in_maps(x, G, v, w)
    res = run_bass_kernel_spmd(nc, in_maps, core_ids=list(range(NCORES)))
    acc = np.zeros((M, B), dtype=np.float64)
    for r in res.results:
        acc += r["out"].astype(np.float64)
    return np.ascontiguousarray(acc.T.astype(np.float32))


# revision 13
# speedup vs baseline: 1.0349x; 1.0349x over previous
"""Trainium2 Bass kernel for nn_ConvexReLU.

Math: out[i,m] = sum_{j,k,l} G[j,k] * x[i,k,l] * (v-w)[j,l,m]

Reassociated as:
    d = v - w                              (host, elementwise)
    T[k,l,m]   = sum_j G[j,k] * d[j,l,m]   (device matmul, 68.7 GFLOP)
    out[i,m]   = sum_{k,l} x[i,k,l] * T[k,l,m]   (device matmul, 17.2 GFLOP)

Sharding: split l (in_dim, 256) across 8 cores (32 each). Each core computes
a full-shape (out_dim, batch) partial; host sums the 8 partials.

Device layout per core:
    g  : (1024 j, 1024 k)        full G, replicated
    d  : (1024 j, 32 l, 128 m)   l-shard of v-w
    xt : (32 l, 128 p, 8 kt, 256 i)  l-shard of x, pre-transposed on host so
                                 each l's tile is contiguous per partition
    out: (128 m, 256 i)          partial of out^T

DMA plan (two HWDGE rings, FIFO each):
    sync  : g chunks (pg=0's critical path), then d for pg=1..7
    scalar: d for pg=0 (per-jc chunks), then ALL x tiles up front —
            paced by xpool buffer reuse, so the last l-group's x lands
            ~40us before its stage-2 instead of being fetched at the end.

Default dtype is bf16 (PE multiplies at fp22 internally, accumulates fp32;
measured rel err ~3e-3).
"""

import os
import sys

import numpy as np

for _p in ("/opt/trn_rl_repo", "/root/.axon_site/_ro/trn_rl_repo"):
    if os.path.isdir(_p) and _p not in sys.path:
        sys.path.insert(0, _p)

import concourse.bass as bass
import concourse.bacc as bacc
import concourse.mybir as mybir
from concourse.bass_utils import run_bass_kernel_spmd
from concourse.tile import TileContext

B, J, K, L, M = 256, 1024, 1024, 256, 128
NCORES = 8
LC = L // NCORES          # 32 l-values per core
NPG = 8                   # l-groups per core
LG = LC // NPG            # 4 l-values per group
NKT = K // 128            # 8 k-tiles
NJC = J // 128            # 8 j-chunks

F32 = mybir.dt.float32
F32R = mybir.dt.float32r
BF16 = mybir.dt.bfloat16

DTYPE = os.environ.get("BASS_KERNEL_DTYPE", "bf16")


def _dtypes(dtype_name: str):
    if dtype_name == "bf16":
        return BF16, BF16
    if dtype_name == "mixed":
        return F32R, BF16
    return F32R, F32R


def build_nc(dtype_name: str = DTYPE) -> bass.Bass:
    gd_dt, s2_dt = _dtypes(dtype_name)

    nc = bacc.Bacc(None, debug=False)

    g = nc.declare_dram_parameter("g", [J, K], gd_dt, isOutput=False)
    d = nc.declare_dram_parameter("d", [J, LC, M], gd_dt, isOutput=False)
    xt = nc.declare_dram_parameter("xt", [LC, 128, NKT, B], s2_dt, isOutput=False)
    out = nc.declare_dram_parameter("out", [M, B], F32, isOutput=True)

    g_r = g.rearrange("(jc p) k -> p jc k", p=128)
    d_r = d.rearrange("(jc p) l m -> p jc (l m)", p=128)

    with TileContext(nc) as tc:
        with (
            tc.tile_pool(name="gpool", bufs=1) as gpool,
            tc.tile_pool(name="dpool", bufs=4) as dpool,
            tc.tile_pool(name="tpool", bufs=3) as tpool,
            tc.tile_pool(name="xpool", bufs=12) as xpool,
            tc.tile_pool(name="opool", bufs=1) as opool,
            tc.tile_pool(name="wupool", bufs=1) as wupool,
            tc.tile_pool(name="ps1", bufs=7, space="PSUM") as ps1,
            tc.tile_pool(name="pso", bufs=1, space="PSUM") as pso,
        ):
            # ---- front DMAs: g on sync, d(pg=0) on scalar. Small first
            # chunks for a fast first matmul, then coarse chunks: each DMA
            # pays ~2us completion latency and the per-engine semaphore-lane
            # rotation is only ~4 deep, so fewer/bigger transfers keep the
            # feed ahead of the PE ----
            g_sb = gpool.tile([128, NJC, K], gd_dt)
            d_sb0 = dpool.tile([128, NJC, LG * M], gd_dt, tag="d")
            nc.sync.dma_start(out=g_sb[:, 0, 0:256], in_=g_r[:, 0, 0:256])
            nc.scalar.dma_start(out=d_sb0[:, 0, :], in_=d_r[:, 0, 0 : LG * M])
            nc.sync.dma_start(out=g_sb[:, 0, 256:], in_=g_r[:, 0, 256:])
            nc.scalar.dma_start(out=d_sb0[:, 1, :], in_=d_r[:, 1, 0 : LG * M])
            nc.sync.dma_start(out=g_sb[:, 1, :], in_=g_r[:, 1, :])
            nc.scalar.dma_start(
                out=d_sb0[:, 2:4, :], in_=d_r[:, 2:4, 0 : LG * M]
            )
            nc.sync.dma_start(out=g_sb[:, 2:4, :], in_=g_r[:, 2:4, :])
            nc.scalar.dma_start(
                out=d_sb0[:, 4:, :], in_=d_r[:, 4:, 0 : LG * M]
            )
            nc.sync.dma_start(out=g_sb[:, 4:6, :], in_=g_r[:, 4:6, :])
            nc.sync.dma_start(out=g_sb[:, 6:, :], in_=g_r[:, 6:, :])

            # ---- d for pg>=1, two halves each so stage-1's jc loop can
            # start on the first half. d(1) goes on the scalar ring ahead of
            # the x stream (it's needed ~17us in, before x); d(2..7) go on
            # sync behind g. dpool bufs=4 lets the dispatches run 3 groups
            # ahead of stage-1 consumption ----
            d_sbs = [d_sb0]
            for pg in range(1, NPG):
                d_sb = dpool.tile([128, NJC, LG * M], gd_dt, tag="d")
                eng = nc.scalar if pg == 1 else nc.sync
                eng.dma_start(
                    out=d_sb[:, 0 : NJC // 2, :],
                    in_=d_r[:, 0 : NJC // 2, pg * LG * M : (pg + 1) * LG * M],
                )
                eng.dma_start(
                    out=d_sb[:, NJC // 2 :, :],
                    in_=d_r[:, NJC // 2 :, pg * LG * M : (pg + 1) * LG * M],
                )
                d_sbs.append(d_sb)

            # ---- ALL x tiles on scalar ring, issued now; xpool bufs=12
            # means at most 3 l-groups are in flight — the ring stalls on the
            # pool-reuse semaphore, which is exactly the pacing we want ----
            x_tiles = {}
            for pg in range(NPG):
                for dl in range(LG):
                    x_sb = xpool.tile(
                        [128, NKT, B], s2_dt, tag="x", name=f"x_{pg}_{dl}"
                    )
                    nc.scalar.dma_start(out=x_sb[:], in_=xt[pg * LG + dl])
                    x_tiles[(pg, dl)] = x_sb

            # ---- HAM warmup: the PE sits idle from the end of its preamble
            # (~6us) until the first DMA lands (~10.8us), and runs at the
            # K=4/8 half-clock gate for its first few us of matmuls. Filling
            # the DMA-wait window with matmuls on a memset tile banks the
            # warmup credit so real matmuls start at full clock. ----
            wu_sb = wupool.tile([128, 512], gd_dt, name="wu")
            nc.vector.memset(wu_sb[:], 1.0)
            wu_ps = ps1.tile([128, LG * M], F32, tag="p1", name="wups")
            for i in range(6):
                nc.tensor.matmul(
                    wu_ps[:],
                    wu_sb[:, 0:128],
                    wu_sb[:],
                    start=(i == 0),
                    stop=(i == 5),
                    skip_group_check=True,
                )

            out_ps = pso.tile([M, B], F32)

            total_mm2 = NPG * LG * NKT
            # kt-groups per stage-1 pass: 6 then 2 stage-1 psum banks live,
            # plus 1 out bank <= 8
            KGROUPS = [(0, 6), (6, 2)]
            KH = 4  # stage-2 kt-group width

            mm2_state = [0]

            def stage2(pg, t_sb):
                # out^T += T^T-slices @ x^T-slices for l-group pg.
                for half in range(NKT // KH):
                    for dl in range(LG):
                        for kt2 in range(KH):
                            kt = half * KH + kt2
                            nc.tensor.matmul(
                                out_ps[:],
                                t_sb[:, kt, dl * M : (dl + 1) * M],
                                x_tiles[(pg, dl)][:, kt, :],
                                start=(mm2_state[0] == 0),
                                stop=(mm2_state[0] == total_mm2 - 1),
                                skip_group_check=True,
                            )
                            mm2_state[0] += 1

            prev = None  # (pg, t_sb) whose stage-2 is pending

            for pg in range(NPG):
                # ---- stage 1: T[k, (l,m)] for this l-group ----
                d_sb = d_sbs[pg]
                t_sb = tpool.tile([128, NKT, LG * M], s2_dt, tag="t")
                for gi, (k0, kn) in enumerate(KGROUPS):
                    p1s = [
                        ps1.tile(
                            [128, LG * M], F32, tag="p1", name=f"p1_{pg}_{gi}_{i}"
                        )
                        for i in range(kn)
                    ]
                    # jc-outer: each (g[jc], d[jc]) pair is fully consumed as
                    # soon as its DMA lands
                    for jc in range(NJC):
                        for kt2 in range(kn):
                            kt = k0 + kt2
                            nc.tensor.matmul(
                                p1s[kt2][:],
                                g_sb[:, jc, kt * 128 : (kt + 1) * 128],
                                d_sb[:, jc, :],
                                start=(jc == 0),
                                stop=(jc == NJC - 1),
                                skip_group_check=True,
                            )
                    for kt2 in range(kn):
                        kt = k0 + kt2
                        nc.vector.tensor_copy(out=t_sb[:, kt, :], in_=p1s[kt2][:])

                # stage-2 lags stage-1 by one l-group
                if prev is not None:
                    stage2(*prev)
                prev = (pg, t_sb)

            stage2(*prev)

            out_sb = opool.tile([M, B], F32)
            nc.vector.tensor_copy(out=out_sb[:], in_=out_ps[:])
            nc.sync.dma_start(out=out[:], in_=out_sb[:])

    nc.finalize()
    return nc


_NC_CACHE: dict[str, bass.Bass] = {}


def _get_nc(dtype_name: str = DTYPE) -> bass.Bass:
    if dtype_name not in _NC_CACHE:
        _NC_CACHE[dtype_name] = build_nc(dtype_name)
    return _NC_CACHE[dtype_name]


def make_in_maps(x, G, v, w, dtype_name: str = DTYPE):
    x = np.asarray(x, dtype=np.float32)
    G = np.asarray(G, dtype=np.float32)
    v = np.asarray(v, dtype=np.float32)
    w = np.asarray(w, dtype=np.float32)

    d_full = v - w  # (J, L, M)

    import ml_dtypes

    if dtype_name == "bf16":
        gd_np, x_np = ml_dtypes.bfloat16, ml_dtypes.bfloat16
    elif dtype_name == "mixed":
        gd_np, x_np = np.float32, ml_dtypes.bfloat16
    else:
        gd_np, x_np = np.float32, np.float32

    G_io = np.ascontiguousarray(G.astype(gd_np))
    in_maps = []
    for c in range(NCORES):
        ls = slice(c * LC, (c + 1) * LC)
        d_c = np.ascontiguousarray(d_full[:, ls, :].astype(gd_np))
        # x (B,K,L) -> xt (LC, 128, NKT, B): xt[l, p, kt, i] = x[i, kt*128+p, l]
        xt_c = (
            x[:, :, ls]
            .transpose(2, 1, 0)                    # (LC, K, B)
            .reshape(LC, NKT, 128, B)
            .transpose(0, 2, 1, 3)                 # (LC, 128, NKT, B)
        )
        xt_c = np.ascontiguousarray(xt_c.astype(x_np))
        in_maps.append({"g": G_io, "d": d_c, "xt": xt_c})
    return in_maps


def kernel(x, G, v, w):
    nc = _get_nc()
    in_maps = make_in_maps(x, G, v, w)
    res = run_bass_kernel_spmd(nc, in_maps, core_ids=list(range(NCORES)))
    acc = np.zeros((M, B), dtype=np.float64)
    for r in res.results:
        acc += r["out"].astype(np.float64)
    return np.ascontiguousarray(acc.T.astype(np.float32))


# revision 14
# speedup vs baseline: 1.0440x; 1.0088x over previous
"""Trainium2 Bass kernel for nn_ConvexReLU.

Math: out[i,m] = sum_{j,k,l} G[j,k] * x[i,k,l] * (v-w)[j,l,m]

Reassociated as:
    d = v - w                              (host, elementwise)
    T[k,l,m]   = sum_j G[j,k] * d[j,l,m]   (device matmul, 68.7 GFLOP)
    out[i,m]   = sum_{k,l} x[i,k,l] * T[k,l,m]   (device matmul, 17.2 GFLOP)

Sharding: split l (in_dim, 256) across 8 cores (32 each). Each core computes
a full-shape (out_dim, batch) partial; host sums the 8 partials.

Device layout per core:
    g  : (1024 j, 1024 k)        full G, replicated
    d  : (1024 j, 32 l, 128 m)   l-shard of v-w
    xt : (32 l, 128 p, 8 kt, 256 i)  l-shard of x, pre-transposed on host so
                                 each l's tile is contiguous per partition
    out: (128 m, 256 i)          partial of out^T

DMA plan (two HWDGE rings, FIFO each):
    sync  : g chunks (pg=0's critical path), then d for pg=1..7
    scalar: d for pg=0 (per-jc chunks), then ALL x tiles up front —
            paced by xpool buffer reuse, so the last l-group's x lands
            ~40us before its stage-2 instead of being fetched at the end.

Default dtype is bf16 (PE multiplies at fp22 internally, accumulates fp32;
measured rel err ~3e-3).
"""

import os
import sys

import numpy as np

for _p in ("/opt/trn_rl_repo", "/root/.axon_site/_ro/trn_rl_repo"):
    if os.path.isdir(_p) and _p not in sys.path:
        sys.path.insert(0, _p)

import concourse.bass as bass
import concourse.bacc as bacc
import concourse.mybir as mybir
from concourse.bass_utils import run_bass_kernel_spmd
from concourse.tile import TileContext

B, J, K, L, M = 256, 1024, 1024, 256, 128
NCORES = 8
LC = L // NCORES          # 32 l-values per core
NPG = 8                   # l-groups per core
LG = LC // NPG            # 4 l-values per group
NKT = K // 128            # 8 k-tiles
NJC = J // 128            # 8 j-chunks

F32 = mybir.dt.float32
F32R = mybir.dt.float32r
BF16 = mybir.dt.bfloat16

DTYPE = os.environ.get("BASS_KERNEL_DTYPE", "bf16")


def _dtypes(dtype_name: str):
    if dtype_name == "bf16":
        return BF16, BF16
    if dtype_name == "mixed":
        return F32R, BF16
    return F32R, F32R


def build_nc(dtype_name: str = DTYPE) -> bass.Bass:
    gd_dt, s2_dt = _dtypes(dtype_name)

    nc = bacc.Bacc(None, debug=False)

    g = nc.declare_dram_parameter("g", [J, K], gd_dt, isOutput=False)
    d = nc.declare_dram_parameter("d", [J, LC, M], gd_dt, isOutput=False)
    xt = nc.declare_dram_parameter("xt", [LC, 128, NKT, B], s2_dt, isOutput=False)
    out = nc.declare_dram_parameter("out", [M, B], F32, isOutput=True)

    g_r = g.rearrange("(jc p) k -> p jc k", p=128)
    d_r = d.rearrange("(jc p) l m -> p jc (l m)", p=128)

    with TileContext(nc) as tc:
        with (
            tc.tile_pool(name="gpool", bufs=1) as gpool,
            tc.tile_pool(name="dpool", bufs=4) as dpool,
            tc.tile_pool(name="tpool", bufs=3) as tpool,
            tc.tile_pool(name="xpool", bufs=12) as xpool,
            tc.tile_pool(name="opool", bufs=1) as opool,
            tc.tile_pool(name="wupool", bufs=1) as wupool,
            tc.tile_pool(name="ps1", bufs=7, space="PSUM") as ps1,
            tc.tile_pool(name="pso", bufs=1, space="PSUM") as pso,
        ):
            # ---- front DMAs: g on sync, d(pg=0) on scalar. Small first
            # chunks for a fast first matmul, then coarse chunks: each DMA
            # pays ~2us completion latency and the per-engine semaphore-lane
            # rotation is only ~4 deep, so fewer/bigger transfers keep the
            # feed ahead of the PE ----
            # per-jc (g, d0) chunk pairs alternating across the two rings:
            # the pair for jc lands every ~0.7us, ahead of the PE's ~1.3us
            # per-jc consumption
            g_sb = gpool.tile([128, NJC, K], gd_dt)
            d_sb0 = dpool.tile([128, NJC, LG * M], gd_dt, tag="d")
            for jc in range(NJC):
                ga = nc.sync if jc % 2 == 0 else nc.scalar
                da = nc.scalar if jc % 2 == 0 else nc.sync
                if jc == 0:
                    ga.dma_start(out=g_sb[:, 0, 0:256], in_=g_r[:, 0, 0:256])
                    da.dma_start(out=d_sb0[:, 0, :], in_=d_r[:, 0, 0 : LG * M])
                    ga.dma_start(out=g_sb[:, 0, 256:], in_=g_r[:, 0, 256:])
                else:
                    ga.dma_start(out=g_sb[:, jc, :], in_=g_r[:, jc, :])
                    da.dma_start(
                        out=d_sb0[:, jc, :], in_=d_r[:, jc, 0 : LG * M]
                    )

            # ---- d for pg>=1, two halves each so stage-1's jc loop can
            # start on the first half. d(1) goes on the scalar ring ahead of
            # the x stream (it's needed ~17us in, before x); d(2..7) go on
            # sync behind g. dpool bufs=4 lets the dispatches run 3 groups
            # ahead of stage-1 consumption ----
            d_sbs = [d_sb0]
            for pg in range(1, NPG):
                d_sb = dpool.tile([128, NJC, LG * M], gd_dt, tag="d")
                eng = nc.scalar if pg == 1 else nc.sync
                eng.dma_start(
                    out=d_sb[:, 0 : NJC // 2, :],
                    in_=d_r[:, 0 : NJC // 2, pg * LG * M : (pg + 1) * LG * M],
                )
                eng.dma_start(
                    out=d_sb[:, NJC // 2 :, :],
                    in_=d_r[:, NJC // 2 :, pg * LG * M : (pg + 1) * LG * M],
                )
                d_sbs.append(d_sb)

            # ---- ALL x tiles on scalar ring, issued now; xpool bufs=12
            # means at most 3 l-groups are in flight — the ring stalls on the
            # pool-reuse semaphore, which is exactly the pacing we want ----
            x_tiles = {}
            for pg in range(NPG):
                for dl in range(LG):
                    x_sb = xpool.tile(
                        [128, NKT, B], s2_dt, tag="x", name=f"x_{pg}_{dl}"
                    )
                    nc.scalar.dma_start(out=x_sb[:], in_=xt[pg * LG + dl])
                    x_tiles[(pg, dl)] = x_sb

            # ---- HAM warmup: the PE sits idle from the end of its preamble
            # (~6us) until the first DMA lands (~10.8us), and runs at the
            # K=4/8 half-clock gate for its first few us of matmuls. Filling
            # the DMA-wait window with matmuls on a memset tile banks the
            # warmup credit so real matmuls start at full clock. ----
            wu_sb = wupool.tile([128, 512], gd_dt, name="wu")
            nc.vector.memset(wu_sb[:], 1.0)
            wu_ps = ps1.tile([128, LG * M], F32, tag="p1", name="wups")
            for i in range(6):
                nc.tensor.matmul(
                    wu_ps[:],
                    wu_sb[:, 0:128],
                    wu_sb[:],
                    start=(i == 0),
                    stop=(i == 5),
                    skip_group_check=True,
                )

            out_ps = pso.tile([M, B], F32)

            total_mm2 = NPG * LG * NKT
            # kt-groups per stage-1 pass: 6 then 2 stage-1 psum banks live,
            # plus 1 out bank <= 8
            KGROUPS = [(0, 6), (6, 2)]
            KH = 4  # stage-2 kt-group width

            mm2_state = [0]

            def stage2(pg, t_sb):
                # out^T += T^T-slices @ x^T-slices for l-group pg.
                for half in range(NKT // KH):
                    for dl in range(LG):
                        for kt2 in range(KH):
                            kt = half * KH + kt2
                            nc.tensor.matmul(
                                out_ps[:],
                                t_sb[:, kt, dl * M : (dl + 1) * M],
                                x_tiles[(pg, dl)][:, kt, :],
                                start=(mm2_state[0] == 0),
                                stop=(mm2_state[0] == total_mm2 - 1),
                                skip_group_check=True,
                            )
                            mm2_state[0] += 1

            prev = None  # (pg, t_sb) whose stage-2 is pending

            for pg in range(NPG):
                # ---- stage 1: T[k, (l,m)] for this l-group ----
                d_sb = d_sbs[pg]
                t_sb = tpool.tile([128, NKT, LG * M], s2_dt, tag="t")
                for gi, (k0, kn) in enumerate(KGROUPS):
                    p1s = [
                        ps1.tile(
                            [128, LG * M], F32, tag="p1", name=f"p1_{pg}_{gi}_{i}"
                        )
                        for i in range(kn)
                    ]
                    # jc-outer: each (g[jc], d[jc]) pair is fully consumed as
                    # soon as its DMA lands
                    for jc in range(NJC):
                        for kt2 in range(kn):
                            kt = k0 + kt2
                            nc.tensor.matmul(
                                p1s[kt2][:],
                                g_sb[:, jc, kt * 128 : (kt + 1) * 128],
                                d_sb[:, jc, :],
                                start=(jc == 0),
                                stop=(jc == NJC - 1),
                                skip_group_check=True,
                            )
                    for kt2 in range(kn):
                        kt = k0 + kt2
                        nc.vector.tensor_copy(out=t_sb[:, kt, :], in_=p1s[kt2][:])

                # stage-2 lags stage-1 by one l-group
                if prev is not None:
                    stage2(*prev)
                prev = (pg, t_sb)

            stage2(*prev)

            out_sb = opool.tile([M, B], F32)
            nc.vector.tensor_copy(out=out_sb[:], in_=out_ps[:])
            nc.sync.dma_start(out=out[:], in_=out_sb[:])

    nc.finalize()
    return nc


_NC_CACHE: dict[str, bass.Bass] = {}


def _get_nc(dtype_name: str = DTYPE) -> bass.Bass:
    if dtype_name not in _NC_CACHE:
        _NC_CACHE[dtype_name] = build_nc(dtype_name)
    return _NC_CACHE[dtype_name]


def make_in_maps(x, G, v, w, dtype_name: str = DTYPE):
    x = np.asarray(x, dtype=np.float32)
    G = np.asarray(G, dtype=np.float32)
    v = np.asarray(v, dtype=np.float32)
    w = np.asarray(w, dtype=np.float32)

    d_full = v - w  # (J, L, M)

    import ml_dtypes

    if dtype_name == "bf16":
        gd_np, x_np = ml_dtypes.bfloat16, ml_dtypes.bfloat16
    elif dtype_name == "mixed":
        gd_np, x_np = np.float32, ml_dtypes.bfloat16
    else:
        gd_np, x_np = np.float32, np.float32

    G_io = np.ascontiguousarray(G.astype(gd_np))
    in_maps = []
    for c in range(NCORES):
        ls = slice(c * LC, (c + 1) * LC)
        d_c = np.ascontiguousarray(d_full[:, ls, :].astype(gd_np))
        # x (B,K,L) -> xt (LC, 128, NKT, B): xt[l, p, kt, i] = x[i, kt*128+p, l]
        xt_c = (
            x[:, :, ls]
            .transpose(2, 1, 0)                    # (LC, K, B)
            .reshape(LC, NKT, 128, B)
            .transpose(0, 2, 1, 3)                 # (LC, 128, NKT, B)
        )
        xt_c = np.ascontiguousarray(xt_c.astype(x_np))
        in_maps.append({"g": G_io, "d": d_c, "xt": xt_c})
    return in_maps


def kernel(x, G, v, w):
    nc = _get_nc()
    in_maps = make_in_maps(x, G, v, w)
    res = run_bass_kernel_spmd(nc, in_maps, core_ids=list(range(NCORES)))
    acc = np.zeros((M, B), dtype=np.float64)
    for r in res.results:
        acc += r["out"].astype(np.float64)
    return np.ascontiguousarray(acc.T.astype(np.float32))


# revision 15
# speedup vs baseline: 1.0539x; 1.0095x over previous
"""Trainium2 Bass kernel for nn_ConvexReLU.

Math: out[i,m] = sum_{j,k,l} G[j,k] * x[i,k,l] * (v-w)[j,l,m]

Reassociated as:
    d = v - w                              (host, elementwise)
    T[k,l,m]   = sum_j G[j,k] * d[j,l,m]   (device matmul, 68.7 GFLOP)
    out[i,m]   = sum_{k,l} x[i,k,l] * T[k,l,m]   (device matmul, 17.2 GFLOP)

Sharding: split l (in_dim, 256) across 8 cores (32 each). Each core computes
a full-shape (out_dim, batch) partial; host sums the 8 partials.

Device layout per core:
    g  : (1024 j, 1024 k)        full G, replicated
    d  : (1024 j, 32 l, 128 m)   l-shard of v-w
    xt : (32 l, 128 p, 8 kt, 256 i)  l-shard of x, pre-transposed on host so
                                 each l's tile is contiguous per partition
    out: (128 m, 256 i)          partial of out^T

DMA plan (two HWDGE rings, FIFO each):
    sync  : g chunks (pg=0's critical path), then d for pg=1..7
    scalar: d for pg=0 (per-jc chunks), then ALL x tiles up front —
            paced by xpool buffer reuse, so the last l-group's x lands
            ~40us before its stage-2 instead of being fetched at the end.

Default dtype is bf16 (PE multiplies at fp22 internally, accumulates fp32;
measured rel err ~3e-3).
"""

import os
import sys

import numpy as np

for _p in ("/opt/trn_rl_repo", "/root/.axon_site/_ro/trn_rl_repo"):
    if os.path.isdir(_p) and _p not in sys.path:
        sys.path.insert(0, _p)

import concourse.bass as bass
import concourse.bacc as bacc
import concourse.mybir as mybir
from concourse.bass_utils import run_bass_kernel_spmd
from concourse.tile import TileContext

B, J, K, L, M = 256, 1024, 1024, 256, 128
NCORES = 8
LC = L // NCORES          # 32 l-values per core
NPG = 8                   # l-groups per core
LG = LC // NPG            # 4 l-values per group
NKT = K // 128            # 8 k-tiles
NJC = J // 128            # 8 j-chunks

F32 = mybir.dt.float32
F32R = mybir.dt.float32r
BF16 = mybir.dt.bfloat16

DTYPE = os.environ.get("BASS_KERNEL_DTYPE", "bf16")


def _dtypes(dtype_name: str):
    if dtype_name == "bf16":
        return BF16, BF16
    if dtype_name == "mixed":
        return F32R, BF16
    return F32R, F32R


def build_nc(dtype_name: str = DTYPE) -> bass.Bass:
    gd_dt, s2_dt = _dtypes(dtype_name)

    nc = bacc.Bacc(None, debug=False)

    g = nc.declare_dram_parameter("g", [J, K], gd_dt, isOutput=False)
    d = nc.declare_dram_parameter("d", [J, LC, M], gd_dt, isOutput=False)
    xt = nc.declare_dram_parameter("xt", [LC, 128, NKT, B], s2_dt, isOutput=False)
    out = nc.declare_dram_parameter("out", [M, B], F32, isOutput=True)

    g_r = g.rearrange("(jc p) k -> p jc k", p=128)
    d_r = d.rearrange("(jc p) l m -> p jc (l m)", p=128)

    with TileContext(nc) as tc:
        with (
            tc.tile_pool(name="gpool", bufs=1) as gpool,
            tc.tile_pool(name="dpool", bufs=4) as dpool,
            tc.tile_pool(name="tpool", bufs=3) as tpool,
            tc.tile_pool(name="xpool", bufs=12) as xpool,
            tc.tile_pool(name="opool", bufs=1) as opool,
            tc.tile_pool(name="wupool", bufs=1) as wupool,
            tc.tile_pool(name="ps1", bufs=7, space="PSUM") as ps1,
            tc.tile_pool(name="pso", bufs=1, space="PSUM") as pso,
        ):
            # ---- front DMAs: g on sync, d(pg=0) on scalar. Small first
            # chunks for a fast first matmul, then coarse chunks: each DMA
            # pays ~2us completion latency and the per-engine semaphore-lane
            # rotation is only ~4 deep, so fewer/bigger transfers keep the
            # feed ahead of the PE ----
            # per-jc (g, d0) chunk pairs alternating across the two rings:
            # the pair for jc lands every ~0.7us, ahead of the PE's ~1.3us
            # per-jc consumption
            g_sb = gpool.tile([128, NJC, K], gd_dt)
            d_sb0 = dpool.tile([128, NJC, LG * M], gd_dt, tag="d")
            for jc in range(NJC):
                ga = nc.sync if jc % 2 == 0 else nc.scalar
                da = nc.scalar if jc % 2 == 0 else nc.sync
                if jc == 0:
                    ga.dma_start(out=g_sb[:, 0, 0:256], in_=g_r[:, 0, 0:256])
                    da.dma_start(out=d_sb0[:, 0, :], in_=d_r[:, 0, 0 : LG * M])
                    ga.dma_start(out=g_sb[:, 0, 256:], in_=g_r[:, 0, 256:])
                else:
                    ga.dma_start(out=g_sb[:, jc, :], in_=g_r[:, jc, :])
                    da.dma_start(
                        out=d_sb0[:, jc, :], in_=d_r[:, jc, 0 : LG * M]
                    )

            # ---- d for pg>=1, two halves each so stage-1's jc loop can
            # start on the first half. d(1) goes on the scalar ring ahead of
            # the x stream (it's needed ~17us in, before x); d(2..7) go on
            # sync behind g. dpool bufs=4 lets the dispatches run 3 groups
            # ahead of stage-1 consumption ----
            d_sbs = [d_sb0]
            for pg in range(1, NPG):
                d_sb = dpool.tile([128, NJC, LG * M], gd_dt, tag="d")
                eng = nc.scalar if pg == 1 else nc.sync
                eng.dma_start(
                    out=d_sb[:, 0 : NJC // 2, :],
                    in_=d_r[:, 0 : NJC // 2, pg * LG * M : (pg + 1) * LG * M],
                )
                eng.dma_start(
                    out=d_sb[:, NJC // 2 :, :],
                    in_=d_r[:, NJC // 2 :, pg * LG * M : (pg + 1) * LG * M],
                )
                d_sbs.append(d_sb)

            # ---- ALL x tiles on scalar ring, issued now; xpool bufs=12
            # means at most 3 l-groups are in flight — the ring stalls on the
            # pool-reuse semaphore, which is exactly the pacing we want ----
            x_tiles = {}
            for pg in range(NPG):
                for dl in range(LG):
                    x_sb = xpool.tile(
                        [128, NKT, B], s2_dt, tag="x", name=f"x_{pg}_{dl}"
                    )
                    nc.scalar.dma_start(out=x_sb[:], in_=xt[pg * LG + dl])
                    x_tiles[(pg, dl)] = x_sb

            # ---- HAM warmup: the PE sits idle from the end of its preamble
            # (~6us) until the first DMA lands (~10.8us), and runs at the
            # K=4/8 half-clock gate for its first few us of matmuls. Filling
            # the DMA-wait window with matmuls on a memset tile banks the
            # warmup credit so real matmuls start at full clock. ----
            wu_sb = wupool.tile([128, 512], gd_dt, name="wu")
            nc.vector.memset(wu_sb[:], 1.0)
            wu_ps = ps1.tile([128, LG * M], F32, tag="p1", name="wups")
            NWU = 10
            for i in range(NWU):
                nc.tensor.matmul(
                    wu_ps[:],
                    wu_sb[:, 0:128],
                    wu_sb[:],
                    start=(i == 0),
                    stop=(i == NWU - 1),
                    skip_group_check=True,
                )

            out_ps = pso.tile([M, B], F32)

            total_mm2 = NPG * LG * NKT
            # kt-groups per stage-1 pass: 6 then 2 stage-1 psum banks live,
            # plus 1 out bank <= 8
            KGROUPS = [(0, 6), (6, 2)]
            KH = 4  # stage-2 kt-group width

            mm2_state = [0]

            def stage2(pg, t_sb):
                # out^T += T^T-slices @ x^T-slices for l-group pg.
                for half in range(NKT // KH):
                    for dl in range(LG):
                        for kt2 in range(KH):
                            kt = half * KH + kt2
                            nc.tensor.matmul(
                                out_ps[:],
                                t_sb[:, kt, dl * M : (dl + 1) * M],
                                x_tiles[(pg, dl)][:, kt, :],
                                start=(mm2_state[0] == 0),
                                stop=(mm2_state[0] == total_mm2 - 1),
                                skip_group_check=True,
                            )
                            mm2_state[0] += 1

            prev = None  # (pg, t_sb) whose stage-2 is pending

            for pg in range(NPG):
                # ---- stage 1: T[k, (l,m)] for this l-group ----
                d_sb = d_sbs[pg]
                t_sb = tpool.tile([128, NKT, LG * M], s2_dt, tag="t")
                for gi, (k0, kn) in enumerate(KGROUPS):
                    p1s = [
                        ps1.tile(
                            [128, LG * M], F32, tag="p1", name=f"p1_{pg}_{gi}_{i}"
                        )
                        for i in range(kn)
                    ]
                    # jc-outer: each (g[jc], d[jc]) pair is fully consumed as
                    # soon as its DMA lands
                    for jc in range(NJC):
                        for kt2 in range(kn):
                            kt = k0 + kt2
                            nc.tensor.matmul(
                                p1s[kt2][:],
                                g_sb[:, jc, kt * 128 : (kt + 1) * 128],
                                d_sb[:, jc, :],
                                start=(jc == 0),
                                stop=(jc == NJC - 1),
                                skip_group_check=True,
                            )
                    for kt2 in range(kn):
                        kt = k0 + kt2
                        nc.vector.tensor_copy(out=t_sb[:, kt, :], in_=p1s[kt2][:])

                # stage-2 lags stage-1 by one l-group
                if prev is not None:
                    stage2(*prev)
                prev = (pg, t_sb)

            stage2(*prev)

            out_sb = opool.tile([M, B], F32)
            nc.vector.tensor_copy(out=out_sb[:], in_=out_ps[:])
            nc.sync.dma_start(out=out[:], in_=out_sb[:])

    nc.finalize()
    return nc


_NC_CACHE: dict[str, bass.Bass] = {}


def _get_nc(dtype_name: str = DTYPE) -> bass.Bass:
    if dtype_name not in _NC_CACHE:
        _NC_CACHE[dtype_name] = build_nc(dtype_name)
    return _NC_CACHE[dtype_name]


def make_in_maps(x, G, v, w, dtype_name: str = DTYPE):
    x = np.asarray(x, dtype=np.float32)
    G = np.asarray(G, dtype=np.float32)
    v = np.asarray(v, dtype=np.float32)
    w = np.asarray(w, dtype=np.float32)

    d_full = v - w  # (J, L, M)

    import ml_dtypes

    if dtype_name == "bf16":
        gd_np, x_np = ml_dtypes.bfloat16, ml_dtypes.bfloat16
    elif dtype_name == "mixed":
        gd_np, x_np = np.float32, ml_dtypes.bfloat16
    else:
        gd_np, x_np = np.float32, np.float32

    G_io = np.ascontiguousarray(G.astype(gd_np))
    in_maps = []
    for c in range(NCORES):
        ls = slice(c * LC, (c + 1) * LC)
        d_c = np.ascontiguousarray(d_full[:, ls, :].astype(gd_np))
        # x (B,K,L) -> xt (LC, 128, NKT, B): xt[l, p, kt, i] = x[i, kt*128+p, l]
        xt_c = (
            x[:, :, ls]
            .transpose(2, 1, 0)                    # (LC, K, B)
            .reshape(LC, NKT, 128, B)
            .transpose(0, 2, 1, 3)                 # (LC, 128, NKT, B)
        )
        xt_c = np.ascontiguousarray(xt_c.astype(x_np))
        in_maps.append({"g": G_io, "d": d_c, "xt": xt_c})
    return in_maps


def kernel(x, G, v, w):
    nc = _get_nc()
    in_maps = make_in_maps(x, G, v, w)
    res = run_bass_kernel_spmd(nc, in_maps, core_ids=list(range(NCORES)))
    acc = np.zeros((M, B), dtype=np.float64)
    for r in res.results:
        acc += r["out"].astype(np.float64)
    return np.ascontiguousarray(acc.T.astype(np.float32))


# revision 17
# speedup vs baseline: 1.0598x; 1.0056x over previous
"""Trainium2 Bass kernel for nn_ConvexReLU.

Math: out[i,m] = sum_{j,k,l} G[j,k] * x[i,k,l] * (v-w)[j,l,m]

Reassociated as:
    d = v - w                              (host, elementwise)
    T[k,l,m]   = sum_j G[j,k] * d[j,l,m]   (device matmul, 68.7 GFLOP)
    out[i,m]   = sum_{k,l} x[i,k,l] * T[k,l,m]   (device matmul, 17.2 GFLOP)

Sharding: split l (in_dim, 256) across 8 cores (32 each). Each core computes
a full-shape (out_dim, batch) partial; host sums the 8 partials.

Device layout per core:
    g  : (1024 j, 1024 k)        full G, replicated
    d  : (1024 j, 32 l, 128 m)   l-shard of v-w
    xt : (32 l, 128 p, 8 kt, 256 i)  l-shard of x, pre-transposed on host so
                                 each l's tile is contiguous per partition
    out: (128 m, 256 i)          partial of out^T

DMA plan (two HWDGE rings, FIFO each):
    sync  : g chunks (pg=0's critical path), then d for pg=1..7
    scalar: d for pg=0 (per-jc chunks), then ALL x tiles up front —
            paced by xpool buffer reuse, so the last l-group's x lands
            ~40us before its stage-2 instead of being fetched at the end.

Default dtype is bf16 (PE multiplies at fp22 internally, accumulates fp32;
measured rel err ~3e-3).
"""

import os
import sys

import numpy as np

for _p in ("/opt/trn_rl_repo", "/root/.axon_site/_ro/trn_rl_repo"):
    if os.path.isdir(_p) and _p not in sys.path:
        sys.path.insert(0, _p)

import concourse.bass as bass
import concourse.bacc as bacc
import concourse.mybir as mybir
from concourse.bass_utils import run_bass_kernel_spmd
from concourse.tile import TileContext

B, J, K, L, M = 256, 1024, 1024, 256, 128
NCORES = 8
LC = L // NCORES          # 32 l-values per core
NPG = 8                   # l-groups per core
LG = LC // NPG            # 4 l-values per group
NKT = K // 128            # 8 k-tiles
NJC = J // 128            # 8 j-chunks

F32 = mybir.dt.float32
F32R = mybir.dt.float32r
BF16 = mybir.dt.bfloat16

DTYPE = os.environ.get("BASS_KERNEL_DTYPE", "bf16")


def _dtypes(dtype_name: str):
    if dtype_name == "bf16":
        return BF16, BF16
    if dtype_name == "mixed":
        return F32R, BF16
    return F32R, F32R


def build_nc(dtype_name: str = DTYPE) -> bass.Bass:
    gd_dt, s2_dt = _dtypes(dtype_name)

    nc = bacc.Bacc(None, debug=False)

    g = nc.declare_dram_parameter("g", [J, K], gd_dt, isOutput=False)
    d = nc.declare_dram_parameter("d", [J, LC, M], gd_dt, isOutput=False)
    xt = nc.declare_dram_parameter("xt", [LC, 128, NKT, B], s2_dt, isOutput=False)
    out = nc.declare_dram_parameter("out", [M, B], BF16, isOutput=True)

    g_r = g.rearrange("(jc p) k -> p jc k", p=128)
    d_r = d.rearrange("(jc p) l m -> p jc (l m)", p=128)

    with TileContext(nc) as tc:
        with (
            tc.tile_pool(name="gpool", bufs=1) as gpool,
            tc.tile_pool(name="dpool", bufs=4) as dpool,
            tc.tile_pool(name="tpool", bufs=3) as tpool,
            tc.tile_pool(name="xpool", bufs=12) as xpool,
            tc.tile_pool(name="opool", bufs=1) as opool,
            tc.tile_pool(name="wupool", bufs=1) as wupool,
            tc.tile_pool(name="ps1", bufs=7, space="PSUM") as ps1,
            tc.tile_pool(name="pso", bufs=1, space="PSUM") as pso,
        ):
            # ---- front DMAs: g on sync, d(pg=0) on scalar. Small first
            # chunks for a fast first matmul, then coarse chunks: each DMA
            # pays ~2us completion latency and the per-engine semaphore-lane
            # rotation is only ~4 deep, so fewer/bigger transfers keep the
            # feed ahead of the PE ----
            # per-jc (g, d0) chunk pairs alternating across the two rings:
            # the pair for jc lands every ~0.7us, ahead of the PE's ~1.3us
            # per-jc consumption
            g_sb = gpool.tile([128, NJC, K], gd_dt)
            d_sb0 = dpool.tile([128, NJC, LG * M], gd_dt, tag="d")
            for jc in range(NJC):
                ga = nc.sync if jc % 2 == 0 else nc.scalar
                da = nc.scalar if jc % 2 == 0 else nc.sync
                if jc == 0:
                    ga.dma_start(out=g_sb[:, 0, 0:256], in_=g_r[:, 0, 0:256])
                    da.dma_start(out=d_sb0[:, 0, :], in_=d_r[:, 0, 0 : LG * M])
                    ga.dma_start(out=g_sb[:, 0, 256:], in_=g_r[:, 0, 256:])
                else:
                    ga.dma_start(out=g_sb[:, jc, :], in_=g_r[:, jc, :])
                    da.dma_start(
                        out=d_sb0[:, jc, :], in_=d_r[:, jc, 0 : LG * M]
                    )

            # ---- d for pg>=1, two halves each so stage-1's jc loop can
            # start on the first half. d(1) goes on the scalar ring ahead of
            # the x stream (it's needed ~17us in, before x); d(2..7) go on
            # sync behind g. dpool bufs=4 lets the dispatches run 3 groups
            # ahead of stage-1 consumption ----
            d_sbs = [d_sb0]
            for pg in range(1, NPG):
                d_sb = dpool.tile([128, NJC, LG * M], gd_dt, tag="d")
                eng = nc.scalar if pg == 1 else nc.sync
                eng.dma_start(
                    out=d_sb[:, 0 : NJC // 2, :],
                    in_=d_r[:, 0 : NJC // 2, pg * LG * M : (pg + 1) * LG * M],
                )
                eng.dma_start(
                    out=d_sb[:, NJC // 2 :, :],
                    in_=d_r[:, NJC // 2 :, pg * LG * M : (pg + 1) * LG * M],
                )
                d_sbs.append(d_sb)

            # ---- ALL x tiles on scalar ring, issued now; xpool bufs=12
            # means at most 3 l-groups are in flight — the ring stalls on the
            # pool-reuse semaphore, which is exactly the pacing we want ----
            x_tiles = {}
            for pg in range(NPG):
                for dl in range(LG):
                    x_sb = xpool.tile(
                        [128, NKT, B], s2_dt, tag="x", name=f"x_{pg}_{dl}"
                    )
                    nc.scalar.dma_start(out=x_sb[:], in_=xt[pg * LG + dl])
                    x_tiles[(pg, dl)] = x_sb

            # ---- HAM warmup: the PE sits idle from the end of its preamble
            # (~6us) until the first DMA lands (~10.8us), and runs at the
            # K=4/8 half-clock gate for its first few us of matmuls. Filling
            # the DMA-wait window with matmuls on a memset tile banks the
            # warmup credit so real matmuls start at full clock. ----
            wu_sb = wupool.tile([128, 512], gd_dt, name="wu")
            nc.vector.memset(wu_sb[:], 1.0)
            wu_ps = ps1.tile([128, LG * M], F32, tag="p1", name="wups")
            NWU = 10
            for i in range(NWU):
                nc.tensor.matmul(
                    wu_ps[:],
                    wu_sb[:, 0:128],
                    wu_sb[:],
                    start=(i == 0),
                    stop=(i == NWU - 1),
                    skip_group_check=True,
                )

            out_ps = pso.tile([M, B], F32)

            total_mm2 = NPG * LG * NKT
            # kt-groups per stage-1 pass: 6 then 2 stage-1 psum banks live,
            # plus 1 out bank <= 8
            KGROUPS = [(0, 6), (6, 2)]
            KH = 4  # stage-2 kt-group width

            mm2_state = [0]

            def stage2(pg, t_sb):
                # out^T += T^T-slices @ x^T-slices for l-group pg.
                for half in range(NKT // KH):
                    for dl in range(LG):
                        for kt2 in range(KH):
                            kt = half * KH + kt2
                            nc.tensor.matmul(
                                out_ps[:],
                                t_sb[:, kt, dl * M : (dl + 1) * M],
                                x_tiles[(pg, dl)][:, kt, :],
                                start=(mm2_state[0] == 0),
                                stop=(mm2_state[0] == total_mm2 - 1),
                                skip_group_check=True,
                            )
                            mm2_state[0] += 1

            prev = None  # (pg, t_sb) whose stage-2 is pending

            for pg in range(NPG):
                # ---- stage 1: T[k, (l,m)] for this l-group ----
                d_sb = d_sbs[pg]
                t_sb = tpool.tile([128, NKT, LG * M], s2_dt, tag="t")
                for gi, (k0, kn) in enumerate(KGROUPS):
                    p1s = [
                        ps1.tile(
                            [128, LG * M], F32, tag="p1", name=f"p1_{pg}_{gi}_{i}"
                        )
                        for i in range(kn)
                    ]
                    # jc-outer: each (g[jc], d[jc]) pair is fully consumed as
                    # soon as its DMA lands
                    for jc in range(NJC):
                        for kt2 in range(kn):
                            kt = k0 + kt2
                            nc.tensor.matmul(
                                p1s[kt2][:],
                                g_sb[:, jc, kt * 128 : (kt + 1) * 128],
                                d_sb[:, jc, :],
                                start=(jc == 0),
                                stop=(jc == NJC - 1),
                                skip_group_check=True,
                            )
                    for kt2 in range(kn):
                        kt = k0 + kt2
                        nc.vector.tensor_copy(out=t_sb[:, kt, :], in_=p1s[kt2][:])

                # stage-2 lags stage-1 by one l-group
                if prev is not None:
                    stage2(*prev)
                prev = (pg, t_sb)

            stage2(*prev)

            out_sb = opool.tile([M, B], BF16)
            nc.vector.tensor_copy(out=out_sb[:], in_=out_ps[:])
            nc.sync.dma_start(out=out[:], in_=out_sb[:])

    nc.finalize()
    return nc


_NC_CACHE: dict[str, bass.Bass] = {}


def _get_nc(dtype_name: str = DTYPE) -> bass.Bass:
    if dtype_name not in _NC_CACHE:
        _NC_CACHE[dtype_name] = build_nc(dtype_name)
    return _NC_CACHE[dtype_name]


def make_in_maps(x, G, v, w, dtype_name: str = DTYPE):
    x = np.asarray(x, dtype=np.float32)
    G = np.asarray(G, dtype=np.float32)
    v = np.asarray(v, dtype=np.float32)
    w = np.asarray(w, dtype=np.float32)

    d_full = v - w  # (J, L, M)

    import ml_dtypes

    if dtype_name == "bf16":
        gd_np, x_np = ml_dtypes.bfloat16, ml_dtypes.bfloat16
    elif dtype_name == "mixed":
        gd_np, x_np = np.float32, ml_dtypes.bfloat16
    else:
        gd_np, x_np = np.float32, np.float32

    G_io = np.ascontiguousarray(G.astype(gd_np))
    in_maps = []
    for c in range(NCORES):
        ls = slice(c * LC, (c + 1) * LC)
        d_c = np.ascontiguousarray(d_full[:, ls, :].astype(gd_np))
        # x (B,K,L) -> xt (LC, 128, NKT, B): xt[l, p, kt, i] = x[i, kt*128+p, l]
        xt_c = (
            x[:, :, ls]
            .transpose(2, 1, 0)                    # (LC, K, B)
            .reshape(LC, NKT, 128, B)
            .transpose(0, 2, 1, 3)                 # (LC, 128, NKT, B)
        )
        xt_c = np.ascontiguousarray(xt_c.astype(x_np))
        in_maps.append({"g": G_io, "d": d_c, "xt": xt_c})
    return in_maps


def kernel(x, G, v, w):
    nc = _get_nc()
    in_maps = make_in_maps(x, G, v, w)
    res = run_bass_kernel_spmd(nc, in_maps, core_ids=list(range(NCORES)))
    acc = np.zeros((M, B), dtype=np.float64)
    for r in res.results:
        acc += r["out"].astype(np.float64)
    return np.ascontiguousarray(acc.T.astype(np.float32))
